# revision 15
# baseline (speedup 1.0000x reference)
"""Self-contained Trainium2 Bass kernel for the MACE-style GNN message-passing
problem (N=20000 nodes, E=320000 edges, C=32 channels, 2 layers + readout).

Sharding: receiver-node-parallel across 8 NeuronCores. Edges are sorted by
receiver on the host; core d owns nodes [2500d, 2500(d+1)) and the edges
pointing into them. Within a core, nodes are tiled 20 x 125; each tile's edges
are padded to 128-edge chunks.

Per-edge message msg[e, (j,c)] = w[e, (l(j),c)] * h[send(e), c] * sh[e, j]:
  - phase A: geometry + radial MLP for BOTH layers (s1 = silu(rad@rW1) cached
    to DRAM), spherical harmonics cached as sh2 (each value duplicated, so the
    j->c broadcast multiplies run in DVE 2x packed mode).
  - per chunk: mm2 (unexpanded l-major rW2, 128 cols) -> B = wc * h_send
    (one 1x TT per 4-chunk quad) -> msg = B (x) sh2 (quad-batched 2x TTs per
    l-block) -> one-hot seg-sum matmul accumulated per node tile.
  - PE is software-pipelined: quad q+1's mm2s are emitted before quad q's
    seg-sums so the tensor engine never stalls on the DVE.
h is exchanged between layers with an AllGather; layer-1 h[senders] uses
eagerly-issued dma_gathers from the replicated h (6 groups in flight).
"""

import math
from contextlib import ExitStack

import ml_dtypes
import numpy as np

N = 20000
E = 320000
C = 32
NCORES = 8
NPC = N // NCORES            # 2500 nodes per core
TILE_NODES = 125
TILES = NPC // TILE_NODES    # 20
R_MAX = 5.0
AVG_NEIGH = 16.0
NUM_LAYERS = 2
L_OF_J = np.array([0, 1, 1, 1, 2, 2, 2, 2, 2, 3, 3, 3, 3, 3, 3, 3])
L_START = [0, 1, 4, 9]       # first j of each l block
L_CNT = [1, 3, 5, 7]
GROUP = 24                   # chunks per hs/onehot stream group
GATHER_AHEAD = 5             # gather groups issued ahead (8 ring slots)

BF16 = ml_dtypes.bfloat16


# ----------------------------------------------------------------- host prep

def _node_permutation(receivers):
    """Balance per-tile edge counts: greedy highest-degree-first into the
    lightest tile. Returns perm (new idx -> original node)."""
    import heapq
    deg = np.bincount(receivers, minlength=N)
    order = np.argsort(-deg, kind="stable")
    ntiles = NCORES * TILES
    heap = [(0, t) for t in range(ntiles)]
    heapq.heapify(heap)
    nodes = [[] for _ in range(ntiles)]
    for n in order:
        while True:
            load, t = heapq.heappop(heap)
            if len(nodes[t]) < TILE_NODES:
                break
        nodes[t].append(n)
        if len(nodes[t]) < TILE_NODES:
            heapq.heappush(heap, (load + int(deg[n]), t))
    return np.array([n for t in nodes for n in t], np.int64)


def _prepare(vectors, embed, rW1, rW2, Wupd, Wro, Wout, node_specie, senders,
             receivers):
    perm = _node_permutation(receivers)          # new idx -> original node
    node_map = np.empty(N, np.int64)             # original node -> new idx
    node_map[perm] = np.arange(N)
    receivers = node_map[receivers].astype(np.int32)
    senders_g = node_map[senders].astype(np.int32)   # for h_full gathers

    order = np.argsort(receivers, kind="stable")
    recv_s = receivers[order]
    tile_of = recv_s // TILE_NODES                       # global tile 0..159
    counts = np.bincount(tile_of, minlength=NCORES * TILES).reshape(NCORES, TILES)
    K_t = (-(-counts // 128)).max(axis=0)                # chunks per tile
    CH = int(K_t.sum())
    CH += (-CH) % 4                                      # quads of 4
    tcs = np.zeros(TILES + 1, np.int64)
    tcs[1:] = np.cumsum(K_t)
    tile_edge_start = np.concatenate([[0], np.cumsum(counts.reshape(-1))])
    EP = CH * 128

    h0 = embed[node_specie].astype(np.float32)           # [N, C]

    per_core = []
    for d in range(NCORES):
        eidx = np.full(EP, -1, np.int64)
        for t in range(TILES):
            gt = d * TILES + t
            s, c = tile_edge_start[gt], counts[d, t]
            dst = int(tcs[t]) * 128
            eidx[dst:dst + c] = order[s:s + c]
        valid = eidx >= 0
        ew = np.where(valid, eidx, 0)

        vec = vectors[ew].astype(np.float32)
        vec[~valid] = np.array([1.0, 0.0, 0.0], np.float32)
        snd = np.where(valid, senders[ew], 0).astype(np.int32)
        sndg = np.where(valid, senders_g[ew], 0).astype(np.int32)
        rloc = receivers[ew] % TILE_NODES

        oh = np.zeros((EP, 128), np.float32)
        vs = np.nonzero(valid)[0]
        oh[vs, rloc[vs]] = 1.0 / AVG_NEIGH
        ohT = (oh.reshape(CH, 128, 128).transpose(1, 0, 2)
               .reshape(128, CH * 128).astype(BF16))

        xs = vec[:, 0].reshape(CH, 128).T.copy()
        ys = vec[:, 1].reshape(CH, 128).T.copy()
        zs = vec[:, 2].reshape(CH, 128).T.copy()

        hs0 = h0[snd].astype(BF16)                       # [EP, 32]
        hs0c = (hs0.reshape(CH, 128, C).transpose(1, 0, 2)
                .reshape(128, CH * C).copy())            # [128, CH*32]

        idx16 = sndg.astype(np.int16).reshape(-1, 16).T   # [16, EP/16]
        idxs = np.tile(idx16, (8, 1)).copy()              # [128, EP/16]

        per_core.append(dict(xs=xs, ys=ys, zs=zs, ohT=ohT, hs0c=hs0c,
                             idxs=idxs))

    # rW2 reordered l-major & unexpanded: col l*32+c <- rW2[:, c*4+l]
    rW2lm = np.empty((NUM_LAYERS, 64, 4 * C), np.float32)
    for li in range(NUM_LAYERS):
        w = rW2[li].reshape(64, C, 4)
        for l in range(4):
            rW2lm[li][:, l * C:(l + 1) * C] = w[:, :, l]
    consts = dict(
        rW1img=np.ascontiguousarray(
            np.concatenate([rW1[0], rW1[1]], axis=1).astype(BF16)),          # [8,128]
        rW2img=np.ascontiguousarray(
            np.concatenate([rW2lm[0], rW2lm[1]], axis=1).astype(BF16)),      # [64,256]
        Wupdimg=np.ascontiguousarray(
            np.concatenate([Wupd[0], Wupd[1]], axis=1).astype(np.float32)),  # [128,64]
        Wro=np.ascontiguousarray(Wro.astype(np.float32)),                    # [32,16]
        Wout=np.ascontiguousarray(Wout.astype(np.float32)),                  # [16,1]
    )
    meta = dict(CH=CH, tcs=tcs, perm=perm)
    return consts, per_core, meta


# ------------------------------------------------------------- bass program

def _build(meta, consts):
    import concourse.bass as bass
    import concourse.bacc as bacc
    import concourse.mybir as mybir
    import concourse.tile as tile
    from concourse.masks import make_identity

    f32 = mybir.dt.float32
    bf16 = mybir.dt.bfloat16
    i16 = mybir.dt.int16
    mult = mybir.AluOpType.mult
    Act = mybir.ActivationFunctionType

    CH = meta["CH"]
    tcs = [int(x) for x in meta["tcs"]]
    CHR = tcs[TILES]             # real (non-pad) chunks
    CH4 = CH // 4
    EP = CH * 128
    NQ = (CHR + 3) // 4
    NGRP = (CHR + GROUP - 1) // GROUP

    nc = bacc.Bacc("TRN2", target_bir_lowering=False, debug=False,
                   num_devices=NCORES)

    # I/O -------------------------------------------------------------------
    xs_d = nc.dram_tensor("xs", [128, CH], f32, kind="ExternalInput")
    ys_d = nc.dram_tensor("ys", [128, CH], f32, kind="ExternalInput")
    zs_d = nc.dram_tensor("zs", [128, CH], f32, kind="ExternalInput")
    ohT_d = nc.dram_tensor("ohT", [128, CH * 128], bf16, kind="ExternalInput")
    hs0c_d = nc.dram_tensor("hs0c", [128, CH * C], bf16, kind="ExternalInput")
    idxs_d = nc.dram_tensor("idxs", [128, EP // 16], i16, kind="ExternalInput")
    out_d = nc.dram_tensor("out", [NPC, 1], f32, kind="ExternalOutput")

    rW1_c = nc.inline_tensor(consts["rW1img"], "rW1c")
    rW2_c = nc.inline_tensor(consts["rW2img"], "rW2c")
    Wupd_c = nc.inline_tensor(consts["Wupdimg"], "Wupdc")
    Wro_c = nc.inline_tensor(consts["Wro"], "Wroc")
    Wout_c = nc.inline_tensor(consts["Wout"], "Woutc")

    s1T_d = nc.dram_tensor("s1T_st", [CH4, 128, 512], bf16)
    h_own = nc.dram_tensor("h_own", [NPC, C], bf16)
    h_small = nc.dram_tensor("h_small", [N, C], bf16)
    # gather rows must be a 256B multiple; only cols 0:32 are ever read, the
    # rest is never written (garbage) — no zero-fill needed.
    # NOTE: not addr_space="Shared" — dma_gather must read it, and gathers
    # from the Shared scratchpad fail at runtime.
    h_full = nc.dram_tensor("h_full", [N, 128], bf16)

    def mkap(base, off, dims):
        """AP over base's tensor: explicit free dims [[stride,count],...]."""
        return bass.AP(base.tensor, base.offset + off,
                       [list(base.ap[0])] + [list(d) for d in dims])

    with TileCtx(nc, tile) as tc, ExitStack() as ctx:
        cpool = ctx.enter_context(tc.tile_pool(name="const", bufs=1))
        shpool = ctx.enter_context(tc.tile_pool(name="shall", bufs=1))
        psA = ctx.enter_context(tc.tile_pool(name="psA", bufs=2, space="PSUM"))

        ident = cpool.tile([128, 128], f32)
        make_identity(nc, ident[:])
        eps_ap = cpool.tile([128, 1], f32)
        nc.gpsimd.memset(eps_ap[:], 1e-12)
        negpi_ap = cpool.tile([128, 1], f32)
        nc.gpsimd.memset(negpi_ap[:], -math.pi)
        rW1_sb = cpool.tile([8, 128], bf16)
        rW2_sb = cpool.tile([64, 256], bf16)
        identb = cpool.tile([128, 128], bf16)
        nc.vector.tensor_copy(out=identb[:], in_=ident[:])
        Wupd_sb = cpool.tile([128, 64], f32)
        Wro_sb = cpool.tile([32, 16], f32)
        Wout_sb = cpool.tile([16, 1], f32)
        nc.sync.dma_start(out=rW1_sb[:], in_=rW1_c[:, :])
        nc.sync.dma_start(out=rW2_sb[:], in_=rW2_c[:, :])
        nc.sync.dma_start(out=Wupd_sb[:], in_=Wupd_c[:, :])
        nc.sync.dma_start(out=Wro_sb[:], in_=Wro_c[:, :])
        nc.sync.dma_start(out=Wout_sb[:], in_=Wout_c[:, :])
        idxs_sb = cpool.tile([128, EP // 16], i16)
        nc.sync.dma_start(out=idxs_sb[:], in_=idxs_d[:, :])

        sh_all = shpool.tile([128, CH, 16], bf16)

        # ---------------- Phase A: per-edge geometry + radial MLP ----------
        with tc.tile_pool(name="bulk", bufs=1) as bpool, \
             tc.tile_pool(name="radcp", bufs=3) as rcpool, \
             tc.tile_pool(name="s1st", bufs=3) as s1pool, \
             tc.tile_pool(name="psW", bufs=2, space="PSUM") as psW:

            xs = bpool.tile([128, CH], f32)
            ys = bpool.tile([128, CH], f32)
            zs = bpool.tile([128, CH], f32)
            nc.sync.dma_start(out=xs[:], in_=xs_d[:, :])
            nc.sync.dma_start(out=ys[:], in_=ys_d[:, :])
            nc.sync.dma_start(out=zs[:], in_=zs_d[:, :])

            x2 = bpool.tile([128, CH], f32)
            r2 = bpool.tile([128, CH], f32)
            nc.vector.tensor_tensor(out=x2[:], in0=xs[:], in1=xs[:], op=mult)
            nc.vector.tensor_tensor(out=r2[:], in0=ys[:], in1=ys[:], op=mult)
            nc.vector.tensor_add(out=r2[:], in0=r2[:], in1=x2[:])
            nc.vector.tensor_tensor(out=x2[:], in0=zs[:], in1=zs[:], op=mult)
            nc.vector.tensor_add(out=r2[:], in0=r2[:], in1=x2[:])
            r = bpool.tile([128, CH], f32)
            nc.scalar.activation(out=r[:], in_=r2[:], func=Act.Sqrt,
                                 bias=eps_ap[:])
            rinv = bpool.tile([128, CH], f32)
            nc.vector.reciprocal(out=rinv[:], in_=r[:])

            # envelope polynomial on t = r / R_MAX
            tq = bpool.tile([128, CH], f32)
            nc.scalar.mul(tq[:], r[:], 1.0 / R_MAX)
            ta = bpool.tile([128, CH], f32)
            nc.vector.tensor_scalar(out=ta[:], in0=tq[:], scalar1=-21.0,
                                    scalar2=48.0, op0=mult,
                                    op1=mybir.AluOpType.add)
            nc.vector.tensor_tensor(out=ta[:], in0=ta[:], in1=tq[:], op=mult)
            nc.vector.tensor_scalar_add(out=ta[:], in0=ta[:], scalar1=-28.0)
            t2 = bpool.tile([128, CH], f32)
            t6 = bpool.tile([128, CH], f32)
            nc.vector.tensor_tensor(out=t2[:], in0=tq[:], in1=tq[:], op=mult)
            nc.vector.tensor_tensor(out=t6[:], in0=t2[:], in1=tq[:], op=mult)
            nc.vector.tensor_tensor(out=t6[:], in0=t6[:], in1=t6[:], op=mult)
            nc.vector.tensor_tensor(out=ta[:], in0=ta[:], in1=t6[:], op=mult)
            nc.vector.tensor_scalar_add(out=ta[:], in0=ta[:], scalar1=1.0)
            mask = bpool.tile([128, CH], f32)
            nc.vector.tensor_scalar(out=mask[:], in0=tq[:], scalar1=1.0,
                                    scalar2=None, op0=mybir.AluOpType.is_lt)
            env = bpool.tile([128, CH], f32)
            nc.vector.tensor_tensor(out=env[:], in0=ta[:], in1=mask[:], op=mult)
            rse = bpool.tile([128, CH], f32)
            nc.vector.tensor_tensor(out=rse[:], in0=rinv[:], in1=env[:], op=mult)
            nc.vector.tensor_scalar_mul(out=rse[:], in0=rse[:],
                                        scalar1=float(np.sqrt(2.0 / R_MAX)))

            u = bpool.tile([128, CH], f32)
            v = bpool.tile([128, CH], f32)
            w = bpool.tile([128, CH], f32)
            nc.vector.tensor_tensor(out=u[:], in0=xs[:], in1=rinv[:], op=mult)
            nc.vector.tensor_tensor(out=v[:], in0=ys[:], in1=rinv[:], op=mult)
            nc.vector.tensor_tensor(out=w[:], in0=zs[:], in1=rinv[:], op=mult)

            # spherical harmonics -> sh_all[:, :, j] (bf16)
            s3, s5, s15 = math.sqrt(3.0), math.sqrt(5.0), math.sqrt(15.0)
            ca = math.sqrt(35.0 / 8.0)
            cb = math.sqrt(105.0)
            cc = math.sqrt(21.0 / 8.0)
            cd = math.sqrt(7.0)
            nc.gpsimd.memset(sh_all[:, :, 0], 1.0)
            nc.vector.tensor_scalar_mul(out=sh_all[:, :, 1], in0=u[:], scalar1=s3)
            nc.vector.tensor_scalar_mul(out=sh_all[:, :, 2], in0=v[:], scalar1=s3)
            nc.vector.tensor_scalar_mul(out=sh_all[:, :, 3], in0=w[:], scalar1=s3)
            xy = bpool.tile([128, CH], f32)
            yz = bpool.tile([128, CH], f32)
            xz = bpool.tile([128, CH], f32)
            xx = bpool.tile([128, CH], f32)
            yy = bpool.tile([128, CH], f32)
            zz = bpool.tile([128, CH], f32)
            nc.vector.tensor_tensor(out=xy[:], in0=u[:], in1=v[:], op=mult)
            nc.vector.tensor_tensor(out=yz[:], in0=v[:], in1=w[:], op=mult)
            nc.vector.tensor_tensor(out=xz[:], in0=u[:], in1=w[:], op=mult)
            nc.vector.tensor_tensor(out=xx[:], in0=u[:], in1=u[:], op=mult)
            nc.vector.tensor_tensor(out=yy[:], in0=v[:], in1=v[:], op=mult)
            nc.vector.tensor_tensor(out=zz[:], in0=w[:], in1=w[:], op=mult)
            nc.vector.tensor_scalar_mul(out=sh_all[:, :, 4], in0=xy[:], scalar1=s15)
            nc.vector.tensor_scalar_mul(out=sh_all[:, :, 5], in0=yz[:], scalar1=s15)
            nc.vector.tensor_scalar(out=sh_all[:, :, 6], in0=zz[:],
                                    scalar1=1.5 * s5, scalar2=-0.5 * s5,
                                    op0=mult, op1=mybir.AluOpType.add)
            nc.vector.tensor_scalar_mul(out=sh_all[:, :, 7], in0=xz[:], scalar1=s15)
            xmy = bpool.tile([128, CH], f32)
            nc.vector.tensor_sub(out=xmy[:], in0=xx[:], in1=yy[:])
            nc.vector.tensor_scalar_mul(out=sh_all[:, :, 8], in0=xmy[:],
                                        scalar1=0.5 * s15)
            tt1 = bpool.tile([128, CH], f32)
            tt2 = bpool.tile([128, CH], f32)
            # j9: a*y*(3xx - yy)
            nc.vector.tensor_scalar_mul(out=tt1[:], in0=xx[:], scalar1=3.0)
            nc.vector.tensor_sub(out=tt1[:], in0=tt1[:], in1=yy[:])
            nc.vector.tensor_tensor(out=tt1[:], in0=tt1[:], in1=v[:], op=mult)
            nc.vector.tensor_scalar_mul(out=sh_all[:, :, 9], in0=tt1[:], scalar1=ca)
            # j10: b*xy*z
            nc.vector.tensor_tensor(out=tt1[:], in0=xy[:], in1=w[:], op=mult)
            nc.vector.tensor_scalar_mul(out=sh_all[:, :, 10], in0=tt1[:], scalar1=cb)
            # t5 = 5zz - 1 (reused j11, j13)
            t5 = bpool.tile([128, CH], f32)
            nc.vector.tensor_scalar(out=t5[:], in0=zz[:], scalar1=5.0,
                                    scalar2=-1.0, op0=mult, op1=mybir.AluOpType.add)
            nc.vector.tensor_tensor(out=tt1[:], in0=t5[:], in1=v[:], op=mult)
            nc.vector.tensor_scalar_mul(out=sh_all[:, :, 11], in0=tt1[:], scalar1=cc)
            # j12: 0.5*d*z*(5zz-3)
            nc.vector.tensor_scalar(out=tt2[:], in0=zz[:], scalar1=5.0,
                                    scalar2=-3.0, op0=mult, op1=mybir.AluOpType.add)
            nc.vector.tensor_tensor(out=tt2[:], in0=tt2[:], in1=w[:], op=mult)
            nc.vector.tensor_scalar_mul(out=sh_all[:, :, 12], in0=tt2[:],
                                        scalar1=0.5 * cd)
            # j13: c*x*(5zz-1)
            nc.vector.tensor_tensor(out=tt1[:], in0=t5[:], in1=u[:], op=mult)
            nc.vector.tensor_scalar_mul(out=sh_all[:, :, 13], in0=tt1[:], scalar1=cc)
            # j14: 0.5*b*z*(xx-yy)
            nc.vector.tensor_tensor(out=tt1[:], in0=xmy[:], in1=w[:], op=mult)
            nc.vector.tensor_scalar_mul(out=sh_all[:, :, 14], in0=tt1[:],
                                        scalar1=0.5 * cb)
            # j15: a*x*(xx-3yy)
            nc.vector.tensor_scalar_mul(out=tt1[:], in0=yy[:], scalar1=3.0)
            nc.vector.tensor_sub(out=tt1[:], in0=xx[:], in1=tt1[:])
            nc.vector.tensor_tensor(out=tt1[:], in0=tt1[:], in1=u[:], op=mult)
            nc.vector.tensor_scalar_mul(out=sh_all[:, :, 15], in0=tt1[:], scalar1=ca)

            # radial features, edge-major, then transpose per chunk to [8,128]
            # sin(n*theta), theta = pi*r/R, via Chebyshev recurrence:
            # s_{n+1} = 2*cos(theta)*s_n - s_{n-1}. Only s_1 and c_1 need the
            # range reduction to the Sin LUT's [-pi, pi].
            radial = bpool.tile([128, CH, 8], bf16)
            ki = bpool.tile([128, CH], mybir.dt.int32)
            kf = bpool.tile([128, CH], f32)

            def lut_sin(out_ap, phase_shift):
                # out = sin(pi*(r/R + phase_shift)), range-reduced
                nc.vector.tensor_scalar(
                    out=out_ap, in0=r[:],
                    scalar1=float(1.0 / (2.0 * R_MAX)),
                    scalar2=0.5 + 0.5 * phase_shift,
                    op0=mult, op1=mybir.AluOpType.add)
                nc.vector.tensor_copy(out=ki[:], in_=out_ap)
                nc.vector.tensor_copy(out=kf[:], in_=ki[:])
                nc.vector.tensor_sub(out=out_ap, in0=out_ap, in1=kf[:])
                nc.vector.tensor_scalar(out=kf[:], in0=out_ap, scalar1=0.0,
                                        scalar2=None,
                                        op0=mybir.AluOpType.is_lt)
                nc.vector.tensor_add(out=out_ap, in0=out_ap, in1=kf[:])
                nc.scalar.activation(out=out_ap, in_=out_ap, func=Act.Sin,
                                     scale=2 * math.pi, bias=negpi_ap[:])

            s_prev = bpool.tile([128, CH], f32)   # sin(n*theta)
            s_prev2 = bpool.tile([128, CH], f32)  # sin((n-1)*theta)
            c2 = bpool.tile([128, CH], f32)       # 2*cos(theta)
            st = bpool.tile([128, CH], f32)
            lut_sin(s_prev[:], 0.0)
            lut_sin(c2[:], 0.5)                   # cos(th) = sin(th + pi/2)
            nc.vector.tensor_scalar_mul(out=c2[:], in0=c2[:], scalar1=2.0)
            nc.vector.tensor_tensor(out=radial[:, :, 0], in0=s_prev[:],
                                    in1=rse[:], op=mult)
            nc.gpsimd.memset(s_prev2[:], 0.0)     # sin(0) = 0
            for nrad in range(1, 8):
                nc.vector.tensor_tensor(out=st[:], in0=c2[:], in1=s_prev[:],
                                        op=mult)
                nc.vector.tensor_sub(out=st[:], in0=st[:], in1=s_prev2[:])
                s_prev2, s_prev, st = s_prev, st, s_prev2
                nc.vector.tensor_tensor(out=radial[:, :, nrad], in0=s_prev[:],
                                        in1=rse[:], op=mult)

            # per quad: transpose 4 chunks -> [8,512], mm1 (both layers),
            # silu, stash to DRAM.  PE-pipelined: mm1(g-1) after transposes(g).
            pend1 = [None]

            def flush_mm1():
                if pend1[0] is None:
                    return
                radsb, g = pend1[0]
                pend1[0] = None
                w1ps = psW.tile([128, 512], f32, tag="w1")
                nc.tensor.matmul(out=w1ps[:], lhsT=rW1_sb[:],
                                 rhs=radsb[:], start=True, stop=True)
                s1T2 = s1pool.tile([128, 512], bf16, tag="s1T2")
                nc.scalar.activation(out=s1T2[:], in_=w1ps[:], func=Act.Silu)
                nc.sync.dma_start(out=s1T_d[g, :, :], in_=s1T2[:])

            for g in range(CH4):
                radps = psA.tile([8, 512], bf16, tag="mps")
                for q in range(4):
                    cchunk = g * 4 + q
                    nc.tensor.transpose(out=radps[:, q * 128:(q + 1) * 128],
                                        in_=radial[:, cchunk, :],
                                        identity=identb[:])
                radsb = rcpool.tile([8, 512], bf16, tag="radsb")
                if g % 2 == 0:
                    nc.vector.tensor_copy(out=radsb[:], in_=radps[:])
                else:
                    nc.scalar.copy(out=radsb[:], in_=radps[:])
                flush_mm1()
                pend1[0] = (radsb, g)
            flush_mm1()

        # ---------------- layers -------------------------------------------
        lpools = {}
        lpools["s1g"] = ctx.enter_context(tc.tile_pool(name="s1g", bufs=2))
        lpools["hs0"] = ctx.enter_context(tc.tile_pool(name="hs0", bufs=2))
        lpools["hs1"] = ctx.enter_context(
            tc.tile_pool(name="hs1", bufs=8))
        lpools["oh"] = ctx.enter_context(tc.tile_pool(name="oh", bufs=2))
        lpools["B"] = ctx.enter_context(tc.tile_pool(name="B", bufs=3))
        lpools["msg"] = ctx.enter_context(tc.tile_pool(name="msg", bufs=4))
        lpools["post"] = ctx.enter_context(tc.tile_pool(name="post", bufs=2))
        ps_wc = ctx.enter_context(tc.tile_pool(name="pswc", bufs=3, space="PSUM"))
        ps_agg = ctx.enter_context(tc.tile_pool(name="psagg", bufs=2, space="PSUM"))

        tile_of_chunk = []
        for t in range(TILES):
            tile_of_chunk += [t] * (tcs[t + 1] - tcs[t])

        hs_tiles = {}

        def issue_gather(g):
            if g >= NGRP:
                return
            g0 = g * GROUP
            gs = min(GROUP, CHR - g0)
            hst = lpools["hs1"].tile([128, GROUP, 128], bf16,
                                     name="hs1g", tag="hs1")
            nc.gpsimd.dma_gather(
                out_ap=hst[:, :gs, :],
                in_ap=h_full[:, :],
                idxs_ap=idxs_sb[:, g0 * 8:(g0 + gs) * 8],
                num_idxs=gs * 128,
                num_idxs_reg=gs * 128,
                elem_size=128,
                # >1024 idxs overflows the 64-desc/engine packet
                single_packet=False,
            )
            hs_tiles[g] = hst

        def emit_layer(layer):
            if layer == 1:
                for g in range(min(GATHER_AHEAD, NGRP)):
                    issue_gather(g)

            state = dict(agg=None, hs=None, oh=None, s1g=None, pend=None)

            def flush_segsum():
                if state["pend"] is None:
                    return
                c0, kk, msg4, oh_t = state["pend"]
                state["pend"] = None
                for k in range(kk):
                    cc = c0 + k
                    ti = tile_of_chunk[cc]
                    if cc == tcs[ti]:
                        state["agg"] = ps_agg.tile([128, 512], f32,
                                                   name="agg", tag="agg")
                    nc.tensor.matmul(
                        out=state["agg"][:],
                        lhsT=oh_t[:, cc % GROUP, :],
                        rhs=mkap(msg4[:], k * 32, [[128, 16], [1, 32]]),
                        start=(cc == tcs[ti]),
                        stop=(cc == tcs[ti + 1] - 1))
                    if cc == tcs[ti + 1] - 1:
                        emit_tile_post(layer, ti, state["agg"])

            for qd in range(NQ):
                c0 = qd * 4
                kk = min(4, CHR - c0)
                if c0 % GROUP == 0:
                    g = c0 // GROUP
                    g0 = c0
                    gs = min(GROUP, CHR - g0)
                    ng = (gs + 3) // 4
                    state["oh"] = lpools["oh"].tile([128, GROUP, 128], bf16,
                                                    name="ohg", tag="oh")
                    nc.sync.dma_start(
                        out=state["oh"][:, :gs, :],
                        in_=ohT_d[:, g0 * 128:(g0 + gs) * 128])
                    # s1 slice for this layer: [64, ng, 512]
                    state["s1g"] = lpools["s1g"].tile([64, GROUP // 4, 512],
                                                      bf16, name="s1g", tag="s1g")
                    sd = s1T_d[:, :, :]
                    nc.sync.dma_start(
                        out=state["s1g"][:, :ng, :],
                        in_=bass.AP(sd.tensor,
                                    sd.offset + (qd * 128 + layer * 64) * 512,
                                    [[512, 64], [128 * 512, ng], [1, 512]]))
                    # hs source
                    if layer == 0:
                        state["hs"] = lpools["hs0"].tile([128, GROUP, C],
                                                         bf16, name="hs0g", tag="hs0")
                        nc.sync.dma_start(
                            out=state["hs"][:, :gs, :],
                            in_=hs0c_d[:, g0 * C:(g0 + gs) * C])
                    else:
                        state["hs"] = hs_tiles.pop(g)
                        issue_gather(g + GATHER_AHEAD)

                qi = (c0 % GROUP) // 4
                # mm2: w = s1 @ rW2 (l-major, unexpanded) -> [128, kk, 128]
                wcps = ps_wc.tile([128, 4, 128], f32, tag="wc")
                for k in range(kk):
                    nc.tensor.matmul(
                        out=wcps[:, k, :],
                        lhsT=state["s1g"][:, qi, k * 128:(k + 1) * 128],
                        rhs=rW2_sb[:, layer * 128:(layer + 1) * 128],
                        start=True, stop=True)

                # previous quad's seg-sums go behind this quad's mm2s so the
                # PE always has runnable work while the DVE builds msg
                flush_segsum()

                # B = w * h_send   [128, kk, (l,c)]
                B4 = lpools["B"].tile([128, 4, 128], bf16, tag="B")
                wq = wcps[:]
                hq = state["hs"][:]
                W = C if layer == 0 else 128
                hoff = (c0 % GROUP) * W
                nc.vector.tensor_tensor(
                    out=mkap(B4[:], 0, [[128, kk], [1, 128]]),
                    in0=mkap(wq, 0, [[128, kk], [1, 128]]),
                    in1=mkap(hq, hoff, [[W, kk], [0, 4], [1, 32]]),
                    op=mult)

                # msg[e, j, k, c] = B[e, k, (l(j),c)] * sh[e, c0+k, j]
                # j-major across the quad: one TT per l-block covers all 4
                # chunks (amortizes DVE per-op overhead; these run 1x anyway
                # because sh broadcasts over c). Seg-sum reads chunk k via a
                # strided rhs AP.
                msg4 = lpools["msg"].tile([128, 16, 4, 32], bf16, tag="msg4")
                mq = msg4[:]
                sa = sh_all[:]
                # l=0 (sh==1): copy (scalar: DVE is busier)
                nc.scalar.copy(
                    out=mkap(mq, 0, [[32, kk], [1, 32]]),
                    in_=mkap(B4[:], 0, [[128, kk], [1, 32]]))
                for l in (1, 2, 3):
                    j0, jl = L_START[l], L_CNT[l]
                    # l=3 (the widest block) runs on the Pool engine during
                    # layer 0, when the Q7 has no gathers to generate
                    eng = nc.gpsimd if (l == 3 and layer == 0) else nc.vector
                    eng.tensor_tensor(
                        out=mkap(mq, j0 * 128,
                                 [[128, jl], [32, kk], [1, 32]]),
                        in0=mkap(B4[:], l * 32,
                                 [[0, jl], [128, kk], [1, 32]]),
                        in1=mkap(sa, c0 * 16 + j0,
                                 [[1, jl], [16, kk], [0, 32]]),
                        op=mult)
                state["pend"] = (c0, kk, msg4, state["oh"])
            flush_segsum()

        def emit_tile_post(layer, t, agg):
            pp = lpools["post"]
            sq = pp.tile([128, 512], f32, tag="sq")
            nc.scalar.activation(out=sq[:], in_=agg[:], func=Act.Square)
            scal = pp.tile([128, 128], f32, tag="scal")
            sq_cj = sq[:].rearrange("p (j c) -> p c j", j=16)
            for li, (j0, j1) in enumerate(((1, 4), (4, 9), (9, 16))):
                nc.vector.tensor_reduce(
                    out=scal[:, 32 + li * 32:64 + li * 32],
                    in_=sq_cj[:, :, j0:j1],
                    axis=mybir.AxisListType.X, op=mybir.AluOpType.add)
            # sqrt(sumsq + 1e-12) in place for cols 32:128
            nc.scalar.activation(out=scal[:, 32:128], in_=scal[:, 32:128],
                                 func=Act.Sqrt, bias=eps_ap[:])
            nc.vector.tensor_copy(out=scal[:, 0:32], in_=agg[:, 0:32])
            sct = psA.tile([128, 128], f32, tag="mps")
            nc.tensor.transpose(out=sct[:], in_=scal[:], identity=ident[:])
            scT = pp.tile([128, 128], f32, tag="scT")
            nc.scalar.copy(out=scT[:], in_=sct[:])
            hps = psA.tile([128, 32], f32, tag="mps")
            nc.tensor.matmul(out=hps[:], lhsT=scT[:],
                             rhs=Wupd_sb[:, layer * 32:(layer + 1) * 32],
                             start=True, stop=True)
            hsb = pp.tile([128, 32], f32, tag="hsb")
            nc.scalar.activation(out=hsb[:], in_=hps[:], func=Act.Silu)
            hsbb = pp.tile([128, 32], bf16, tag="hsbb")
            nc.vector.tensor_copy(out=hsbb[:], in_=hsb[:])
            nc.sync.dma_start(out=h_own[t * 125:(t + 1) * 125, :],
                              in_=hsbb[:125, :])
            if layer == 1:
                htp = psA.tile([32, 128], f32, tag="mps")
                nc.tensor.transpose(out=htp[:], in_=hsb[:, :], identity=ident[:])
                hT = pp.tile([32, 128], f32, tag="hT")
                nc.scalar.copy(out=hT[:], in_=htp[:])
                r1p = psA.tile([16, 128], f32, tag="mps")
                nc.tensor.matmul(out=r1p[:], lhsT=Wro_sb[:], rhs=hT[:],
                                 start=True, stop=True)
                r1 = pp.tile([16, 128], f32, tag="r1")
                nc.scalar.activation(out=r1[:], in_=r1p[:], func=Act.Silu)
                op_ = psA.tile([1, 128], f32, tag="mps")
                nc.tensor.matmul(out=op_[:], lhsT=Wout_sb[:], rhs=r1[:],
                                 start=True, stop=True)
                osb = pp.tile([1, 128], f32, tag="osb")
                nc.vector.tensor_copy(out=osb[:], in_=op_[:])
                nc.sync.dma_start(out=out_d[t * 125:(t + 1) * 125, :],
                                  in_=osb[:, :125])

        emit_layer(0)
        nc.gpsimd.collective_compute(
            "AllGather", mybir.AluOpType.bypass,
            replica_groups=[list(range(NCORES))],
            ins=[h_own[:, :]], outs=[h_small[:, :]])
        # spread the 32-wide rows into the 256B-aligned gather layout
        hf = h_full[:, :]
        hs_ = h_small[:, :]
        nc.sync.dma_start(
            out=bass.AP(hf.tensor, 0, [[128, N], [1, C]]),
            in_=bass.AP(hs_.tensor, 0, [[C, N], [1, C]]))
        emit_layer(1)

    nc.compile()
    return nc


class TileCtx:
    """thin wrapper so _build doesn't import tile at module scope"""
    def __init__(self, nc, tile_mod):
        self._tc = tile_mod.TileContext(nc)

    def __enter__(self):
        return self._tc.__enter__()

    def __exit__(self, *a):
        return self._tc.__exit__(*a)


# ------------------------------------------------------------------ runner

def kernel(**inputs):
    inputs = {k: np.asarray(v) for k, v in inputs.items()}
    consts, per_core, meta = _prepare(**inputs)
    nc = _build(meta, consts)

    from concourse.bass_utils import run_bass_kernel_spmd
    in_maps = []
    for d in range(NCORES):
        pc = per_core[d]
        in_maps.append(dict(
            xs=pc["xs"], ys=pc["ys"], zs=pc["zs"],
            ohT=pc["ohT"], hs0c=pc["hs0c"], idxs=pc["idxs"],
        ))
    import os
    trace = bool(int(os.environ.get("KBENCH_TRACE", "0")))
    if trace:
        trace = _ensure_ntff_hook()
    res = run_bass_kernel_spmd(nc, in_maps, core_ids=list(range(NCORES)),
                               trace=trace)
    if trace and res.exec_time_ns is not None:
        print(f"HW exec time: {res.exec_time_ns} ns")
        kernel.last_exec_time_ns = res.exec_time_ns
        kernel.last_trace = res.instructions_and_trace
    out = np.concatenate([res.results[d]["out"] for d in range(NCORES)], axis=0)
    full = np.empty_like(out)
    full[meta["perm"]] = out
    return full


kernel.last_exec_time_ns = None
kernel.last_trace = None


def _ensure_ntff_hook():
    """Make trace=True work when the image's antenv lacks axon_hooks."""
    import sys
    import types
    try:
        from antenv.axon_hooks import get_axon_ntff_profile_hook  # noqa: F401
        return True
    except ImportError:
        pass
    try:
        import antenv
        from trn_agent_boot.trn_boot import _ntff_profile_via_ctypes
        hook = _ntff_profile_via_ctypes("/opt/axon/libaxon_pjrt.so")
        m = types.ModuleType("antenv.axon_hooks")
        _state = {"h": hook}
        m.set_axon_ntff_profile_hook = lambda h: _state.__setitem__("h", h)
        m.get_axon_ntff_profile_hook = lambda: _state["h"]
        sys.modules["antenv.axon_hooks"] = m
        antenv.axon_hooks = m
        return hook is not None
    except Exception:
        return False


# revision 16
# speedup vs baseline: 1.0815x; 1.0815x over previous
"""Self-contained Trainium2 Bass kernel for the MACE-style GNN message-passing
problem (N=20000 nodes, E=320000 edges, C=32 channels, 2 layers + readout).

Sharding: receiver-node-parallel across 8 NeuronCores. Edges are sorted by
receiver on the host; core d owns nodes [2500d, 2500(d+1)) and the edges
pointing into them. Within a core, nodes are tiled 20 x 125; each tile's edges
are padded to 128-edge chunks.

Per-edge message msg[e, (j,c)] = w[e, (l(j),c)] * h[send(e), c] * sh[e, j]:
  - phase A: geometry + radial MLP for BOTH layers (s1 = silu(rad@rW1) cached
    to DRAM), spherical harmonics cached as sh2 (each value duplicated, so the
    j->c broadcast multiplies run in DVE 2x packed mode).
  - per chunk: mm2 (unexpanded l-major rW2, 128 cols) -> B = wc * h_send
    (one 1x TT per 4-chunk quad) -> msg = B (x) sh2 (quad-batched 2x TTs per
    l-block) -> one-hot seg-sum matmul accumulated per node tile.
  - PE is software-pipelined: quad q+1's mm2s are emitted before quad q's
    seg-sums so the tensor engine never stalls on the DVE.
h is exchanged between layers with an AllGather; layer-1 h[senders] uses
eagerly-issued dma_gathers from the replicated h (6 groups in flight).
"""

import math
from contextlib import ExitStack

import ml_dtypes
import numpy as np

N = 20000
E = 320000
C = 32
NCORES = 8
NPC = N // NCORES            # 2500 nodes per core
TILE_NODES = 125
TILES = NPC // TILE_NODES    # 20
R_MAX = 5.0
AVG_NEIGH = 16.0
NUM_LAYERS = 2
L_OF_J = np.array([0, 1, 1, 1, 2, 2, 2, 2, 2, 3, 3, 3, 3, 3, 3, 3])
L_START = [0, 1, 4, 9]       # first j of each l block
L_CNT = [1, 3, 5, 7]
GROUP = 24                   # chunks per hs/onehot stream group
GATHER_AHEAD = 5             # gather groups issued ahead (8 ring slots)

BF16 = ml_dtypes.bfloat16


# ----------------------------------------------------------------- host prep

def _node_permutation(receivers):
    """Balance per-tile edge counts: greedy highest-degree-first into the
    lightest tile. Returns perm (new idx -> original node)."""
    import heapq
    deg = np.bincount(receivers, minlength=N)
    order = np.argsort(-deg, kind="stable")
    ntiles = NCORES * TILES
    heap = [(0, t) for t in range(ntiles)]
    heapq.heapify(heap)
    nodes = [[] for _ in range(ntiles)]
    for n in order:
        while True:
            load, t = heapq.heappop(heap)
            if len(nodes[t]) < TILE_NODES:
                break
        nodes[t].append(n)
        if len(nodes[t]) < TILE_NODES:
            heapq.heappush(heap, (load + int(deg[n]), t))
    return np.array([n for t in nodes for n in t], np.int64)


def _prepare(vectors, embed, rW1, rW2, Wupd, Wro, Wout, node_specie, senders,
             receivers):
    perm = _node_permutation(receivers)          # new idx -> original node
    node_map = np.empty(N, np.int64)             # original node -> new idx
    node_map[perm] = np.arange(N)
    receivers = node_map[receivers].astype(np.int32)
    senders_g = node_map[senders].astype(np.int32)   # for h_full gathers

    order = np.argsort(receivers, kind="stable")
    recv_s = receivers[order]
    tile_of = recv_s // TILE_NODES                       # global tile 0..159
    counts = np.bincount(tile_of, minlength=NCORES * TILES).reshape(NCORES, TILES)
    K_t = (-(-counts // 128)).max(axis=0)                # chunks per tile
    CH = int(K_t.sum())
    CH += (-CH) % 4                                      # quads of 4
    tcs = np.zeros(TILES + 1, np.int64)
    tcs[1:] = np.cumsum(K_t)
    tile_edge_start = np.concatenate([[0], np.cumsum(counts.reshape(-1))])
    EP = CH * 128

    h0 = embed[node_specie].astype(np.float32)           # [N, C]

    per_core = []
    for d in range(NCORES):
        eidx = np.full(EP, -1, np.int64)
        for t in range(TILES):
            gt = d * TILES + t
            s, c = tile_edge_start[gt], counts[d, t]
            dst = int(tcs[t]) * 128
            eidx[dst:dst + c] = order[s:s + c]
        valid = eidx >= 0
        ew = np.where(valid, eidx, 0)

        vec = vectors[ew].astype(np.float32)
        vec[~valid] = np.array([1.0, 0.0, 0.0], np.float32)
        snd = np.where(valid, senders[ew], 0).astype(np.int32)
        sndg = np.where(valid, senders_g[ew], 0).astype(np.int32)
        rloc = receivers[ew] % TILE_NODES

        oh = np.zeros((EP, 128), np.float32)
        vs = np.nonzero(valid)[0]
        oh[vs, rloc[vs]] = 1.0 / AVG_NEIGH
        ohT = (oh.reshape(CH, 128, 128).transpose(1, 0, 2)
               .reshape(128, CH * 128).astype(BF16))

        xs = vec[:, 0].reshape(CH, 128).T.copy()
        ys = vec[:, 1].reshape(CH, 128).T.copy()
        zs = vec[:, 2].reshape(CH, 128).T.copy()

        hs0 = h0[snd].astype(BF16)                       # [EP, 32]
        hs0c = (hs0.reshape(CH, 128, C).transpose(1, 0, 2)
                .reshape(128, CH * C).copy())            # [128, CH*32]

        idx16 = sndg.astype(np.int16).reshape(-1, 16).T   # [16, EP/16]
        idxs = np.tile(idx16, (8, 1)).copy()              # [128, EP/16]

        per_core.append(dict(xs=xs, ys=ys, zs=zs, ohT=ohT, hs0c=hs0c,
                             idxs=idxs))

    # rW2 reordered l-major & unexpanded: col l*32+c <- rW2[:, c*4+l]
    rW2lm = np.empty((NUM_LAYERS, 64, 4 * C), np.float32)
    for li in range(NUM_LAYERS):
        w = rW2[li].reshape(64, C, 4)
        for l in range(4):
            rW2lm[li][:, l * C:(l + 1) * C] = w[:, :, l]
    consts = dict(
        rW1img=np.ascontiguousarray(
            np.concatenate([rW1[0], rW1[1]], axis=1).astype(BF16)),          # [8,128]
        rW2img=np.ascontiguousarray(
            np.concatenate([rW2lm[0], rW2lm[1]], axis=1).astype(BF16)),      # [64,256]
        Wupdimg=np.ascontiguousarray(
            np.concatenate([Wupd[0], Wupd[1]], axis=1).astype(np.float32)),  # [128,64]
        Wro=np.ascontiguousarray(Wro.astype(np.float32)),                    # [32,16]
        Wout=np.ascontiguousarray(Wout.astype(np.float32)),                  # [16,1]
    )
    meta = dict(CH=CH, tcs=tcs, perm=perm)
    return consts, per_core, meta


# ------------------------------------------------------------- bass program

def _build(meta, consts):
    import concourse.bass as bass
    import concourse.bacc as bacc
    import concourse.mybir as mybir
    import concourse.tile as tile
    from concourse.masks import make_identity

    f32 = mybir.dt.float32
    bf16 = mybir.dt.bfloat16
    i16 = mybir.dt.int16
    mult = mybir.AluOpType.mult
    Act = mybir.ActivationFunctionType

    CH = meta["CH"]
    tcs = [int(x) for x in meta["tcs"]]
    CHR = tcs[TILES]             # real (non-pad) chunks
    CH4 = CH // 4
    EP = CH * 128
    NQ = (CHR + 3) // 4
    NGRP = (CHR + GROUP - 1) // GROUP

    nc = bacc.Bacc("TRN2", target_bir_lowering=False, debug=False,
                   num_devices=NCORES)

    # I/O -------------------------------------------------------------------
    xs_d = nc.dram_tensor("xs", [128, CH], f32, kind="ExternalInput")
    ys_d = nc.dram_tensor("ys", [128, CH], f32, kind="ExternalInput")
    zs_d = nc.dram_tensor("zs", [128, CH], f32, kind="ExternalInput")
    ohT_d = nc.dram_tensor("ohT", [128, CH * 128], bf16, kind="ExternalInput")
    hs0c_d = nc.dram_tensor("hs0c", [128, CH * C], bf16, kind="ExternalInput")
    idxs_d = nc.dram_tensor("idxs", [128, EP // 16], i16, kind="ExternalInput")
    out_d = nc.dram_tensor("out", [NPC, 1], f32, kind="ExternalOutput")

    rW1_c = nc.inline_tensor(consts["rW1img"], "rW1c")
    rW2_c = nc.inline_tensor(consts["rW2img"], "rW2c")
    Wupd_c = nc.inline_tensor(consts["Wupdimg"], "Wupdc")
    Wro_c = nc.inline_tensor(consts["Wro"], "Wroc")
    Wout_c = nc.inline_tensor(consts["Wout"], "Woutc")

    s1T_d = nc.dram_tensor("s1T_st", [CH4, 128, 512], bf16)
    h_own = nc.dram_tensor("h_own", [NPC, C], bf16)
    h_small = nc.dram_tensor("h_small", [N, C], bf16)
    # gather rows must be a 256B multiple; only cols 0:32 are ever read, the
    # rest is never written (garbage) — no zero-fill needed.
    # NOTE: not addr_space="Shared" — dma_gather must read it, and gathers
    # from the Shared scratchpad fail at runtime.
    h_full = nc.dram_tensor("h_full", [N, 128], bf16)

    def mkap(base, off, dims):
        """AP over base's tensor: explicit free dims [[stride,count],...]."""
        return bass.AP(base.tensor, base.offset + off,
                       [list(base.ap[0])] + [list(d) for d in dims])

    with TileCtx(nc, tile) as tc, ExitStack() as ctx:
        cpool = ctx.enter_context(tc.tile_pool(name="const", bufs=1))
        shpool = ctx.enter_context(tc.tile_pool(name="shall", bufs=1))
        psA = ctx.enter_context(tc.tile_pool(name="psA", bufs=2, space="PSUM"))

        ident = cpool.tile([128, 128], f32)
        make_identity(nc, ident[:])
        eps_ap = cpool.tile([128, 1], f32)
        nc.gpsimd.memset(eps_ap[:], 1e-12)
        negpi_ap = cpool.tile([128, 1], f32)
        nc.gpsimd.memset(negpi_ap[:], -math.pi)
        rW1_sb = cpool.tile([8, 128], bf16)
        rW2_sb = cpool.tile([64, 256], bf16)
        identb = cpool.tile([128, 128], bf16)
        nc.vector.tensor_copy(out=identb[:], in_=ident[:])
        Wupd_sb = cpool.tile([128, 64], f32)
        Wro_sb = cpool.tile([32, 16], f32)
        Wout_sb = cpool.tile([16, 1], f32)
        nc.sync.dma_start(out=rW1_sb[:], in_=rW1_c[:, :])
        nc.sync.dma_start(out=rW2_sb[:], in_=rW2_c[:, :])
        nc.sync.dma_start(out=Wupd_sb[:], in_=Wupd_c[:, :])
        nc.sync.dma_start(out=Wro_sb[:], in_=Wro_c[:, :])
        nc.sync.dma_start(out=Wout_sb[:], in_=Wout_c[:, :])
        idxs_sb = cpool.tile([128, EP // 16], i16)
        nc.sync.dma_start(out=idxs_sb[:], in_=idxs_d[:, :])

        sh_all = shpool.tile([128, CH, 16], bf16)

        # ---------------- Phase A: per-edge geometry + radial MLP ----------
        with tc.tile_pool(name="bulk", bufs=1) as bpool, \
             tc.tile_pool(name="radcp", bufs=3) as rcpool, \
             tc.tile_pool(name="s1st", bufs=3) as s1pool, \
             tc.tile_pool(name="psW", bufs=2, space="PSUM") as psW:

            xs = bpool.tile([128, CH], f32)
            ys = bpool.tile([128, CH], f32)
            zs = bpool.tile([128, CH], f32)
            nc.sync.dma_start(out=xs[:], in_=xs_d[:, :])
            nc.sync.dma_start(out=ys[:], in_=ys_d[:, :])
            nc.sync.dma_start(out=zs[:], in_=zs_d[:, :])

            x2 = bpool.tile([128, CH], f32)
            r2 = bpool.tile([128, CH], f32)
            nc.vector.tensor_tensor(out=x2[:], in0=xs[:], in1=xs[:], op=mult)
            nc.vector.tensor_tensor(out=r2[:], in0=ys[:], in1=ys[:], op=mult)
            nc.vector.tensor_add(out=r2[:], in0=r2[:], in1=x2[:])
            nc.vector.tensor_tensor(out=x2[:], in0=zs[:], in1=zs[:], op=mult)
            nc.vector.tensor_add(out=r2[:], in0=r2[:], in1=x2[:])
            r = bpool.tile([128, CH], f32)
            nc.scalar.activation(out=r[:], in_=r2[:], func=Act.Sqrt,
                                 bias=eps_ap[:])
            rinv = bpool.tile([128, CH], f32)
            nc.vector.reciprocal(out=rinv[:], in_=r[:])

            # envelope polynomial on t = r / R_MAX
            tq = bpool.tile([128, CH], f32)
            nc.scalar.mul(tq[:], r[:], 1.0 / R_MAX)
            ta = bpool.tile([128, CH], f32)
            nc.vector.tensor_scalar(out=ta[:], in0=tq[:], scalar1=-21.0,
                                    scalar2=48.0, op0=mult,
                                    op1=mybir.AluOpType.add)
            nc.vector.tensor_tensor(out=ta[:], in0=ta[:], in1=tq[:], op=mult)
            nc.vector.tensor_scalar_add(out=ta[:], in0=ta[:], scalar1=-28.0)
            t2 = bpool.tile([128, CH], f32)
            t6 = bpool.tile([128, CH], f32)
            nc.vector.tensor_tensor(out=t2[:], in0=tq[:], in1=tq[:], op=mult)
            nc.vector.tensor_tensor(out=t6[:], in0=t2[:], in1=tq[:], op=mult)
            nc.vector.tensor_tensor(out=t6[:], in0=t6[:], in1=t6[:], op=mult)
            nc.vector.tensor_tensor(out=ta[:], in0=ta[:], in1=t6[:], op=mult)
            nc.vector.tensor_scalar_add(out=ta[:], in0=ta[:], scalar1=1.0)
            mask = bpool.tile([128, CH], f32)
            nc.vector.tensor_scalar(out=mask[:], in0=tq[:], scalar1=1.0,
                                    scalar2=None, op0=mybir.AluOpType.is_lt)
            env = bpool.tile([128, CH], f32)
            nc.vector.tensor_tensor(out=env[:], in0=ta[:], in1=mask[:], op=mult)
            rse = bpool.tile([128, CH], f32)
            nc.vector.tensor_tensor(out=rse[:], in0=rinv[:], in1=env[:], op=mult)
            nc.vector.tensor_scalar_mul(out=rse[:], in0=rse[:],
                                        scalar1=float(np.sqrt(2.0 / R_MAX)))

            u = bpool.tile([128, CH], f32)
            v = bpool.tile([128, CH], f32)
            w = bpool.tile([128, CH], f32)
            nc.vector.tensor_tensor(out=u[:], in0=xs[:], in1=rinv[:], op=mult)
            nc.vector.tensor_tensor(out=v[:], in0=ys[:], in1=rinv[:], op=mult)
            nc.vector.tensor_tensor(out=w[:], in0=zs[:], in1=rinv[:], op=mult)

            # spherical harmonics -> sh_all[:, :, j] (bf16)
            s3, s5, s15 = math.sqrt(3.0), math.sqrt(5.0), math.sqrt(15.0)
            ca = math.sqrt(35.0 / 8.0)
            cb = math.sqrt(105.0)
            cc = math.sqrt(21.0 / 8.0)
            cd = math.sqrt(7.0)
            nc.gpsimd.memset(sh_all[:, :, 0], 1.0)
            nc.vector.tensor_scalar_mul(out=sh_all[:, :, 1], in0=u[:], scalar1=s3)
            nc.vector.tensor_scalar_mul(out=sh_all[:, :, 2], in0=v[:], scalar1=s3)
            nc.vector.tensor_scalar_mul(out=sh_all[:, :, 3], in0=w[:], scalar1=s3)
            xy = bpool.tile([128, CH], f32)
            yz = bpool.tile([128, CH], f32)
            xz = bpool.tile([128, CH], f32)
            xx = bpool.tile([128, CH], f32)
            yy = bpool.tile([128, CH], f32)
            zz = bpool.tile([128, CH], f32)
            nc.vector.tensor_tensor(out=xy[:], in0=u[:], in1=v[:], op=mult)
            nc.vector.tensor_tensor(out=yz[:], in0=v[:], in1=w[:], op=mult)
            nc.vector.tensor_tensor(out=xz[:], in0=u[:], in1=w[:], op=mult)
            nc.vector.tensor_tensor(out=xx[:], in0=u[:], in1=u[:], op=mult)
            nc.vector.tensor_tensor(out=yy[:], in0=v[:], in1=v[:], op=mult)
            nc.vector.tensor_tensor(out=zz[:], in0=w[:], in1=w[:], op=mult)
            nc.vector.tensor_scalar_mul(out=sh_all[:, :, 4], in0=xy[:], scalar1=s15)
            nc.vector.tensor_scalar_mul(out=sh_all[:, :, 5], in0=yz[:], scalar1=s15)
            nc.vector.tensor_scalar(out=sh_all[:, :, 6], in0=zz[:],
                                    scalar1=1.5 * s5, scalar2=-0.5 * s5,
                                    op0=mult, op1=mybir.AluOpType.add)
            nc.vector.tensor_scalar_mul(out=sh_all[:, :, 7], in0=xz[:], scalar1=s15)
            xmy = bpool.tile([128, CH], f32)
            nc.vector.tensor_sub(out=xmy[:], in0=xx[:], in1=yy[:])
            nc.vector.tensor_scalar_mul(out=sh_all[:, :, 8], in0=xmy[:],
                                        scalar1=0.5 * s15)
            tt1 = bpool.tile([128, CH], f32)
            tt2 = bpool.tile([128, CH], f32)
            # j9: a*y*(3xx - yy)
            nc.vector.tensor_scalar_mul(out=tt1[:], in0=xx[:], scalar1=3.0)
            nc.vector.tensor_sub(out=tt1[:], in0=tt1[:], in1=yy[:])
            nc.vector.tensor_tensor(out=tt1[:], in0=tt1[:], in1=v[:], op=mult)
            nc.vector.tensor_scalar_mul(out=sh_all[:, :, 9], in0=tt1[:], scalar1=ca)
            # j10: b*xy*z
            nc.vector.tensor_tensor(out=tt1[:], in0=xy[:], in1=w[:], op=mult)
            nc.vector.tensor_scalar_mul(out=sh_all[:, :, 10], in0=tt1[:], scalar1=cb)
            # t5 = 5zz - 1 (reused j11, j13)
            t5 = bpool.tile([128, CH], f32)
            nc.vector.tensor_scalar(out=t5[:], in0=zz[:], scalar1=5.0,
                                    scalar2=-1.0, op0=mult, op1=mybir.AluOpType.add)
            nc.vector.tensor_tensor(out=tt1[:], in0=t5[:], in1=v[:], op=mult)
            nc.vector.tensor_scalar_mul(out=sh_all[:, :, 11], in0=tt1[:], scalar1=cc)
            # j12: 0.5*d*z*(5zz-3)
            nc.vector.tensor_scalar(out=tt2[:], in0=zz[:], scalar1=5.0,
                                    scalar2=-3.0, op0=mult, op1=mybir.AluOpType.add)
            nc.vector.tensor_tensor(out=tt2[:], in0=tt2[:], in1=w[:], op=mult)
            nc.vector.tensor_scalar_mul(out=sh_all[:, :, 12], in0=tt2[:],
                                        scalar1=0.5 * cd)
            # j13: c*x*(5zz-1)
            nc.vector.tensor_tensor(out=tt1[:], in0=t5[:], in1=u[:], op=mult)
            nc.vector.tensor_scalar_mul(out=sh_all[:, :, 13], in0=tt1[:], scalar1=cc)
            # j14: 0.5*b*z*(xx-yy)
            nc.vector.tensor_tensor(out=tt1[:], in0=xmy[:], in1=w[:], op=mult)
            nc.vector.tensor_scalar_mul(out=sh_all[:, :, 14], in0=tt1[:],
                                        scalar1=0.5 * cb)
            # j15: a*x*(xx-3yy)
            nc.vector.tensor_scalar_mul(out=tt1[:], in0=yy[:], scalar1=3.0)
            nc.vector.tensor_sub(out=tt1[:], in0=xx[:], in1=tt1[:])
            nc.vector.tensor_tensor(out=tt1[:], in0=tt1[:], in1=u[:], op=mult)
            nc.vector.tensor_scalar_mul(out=sh_all[:, :, 15], in0=tt1[:], scalar1=ca)

            # radial features, edge-major, then transpose per chunk to [8,128]
            radial = bpool.tile([128, CH, 8], bf16)
            sinb = bpool.tile([128, CH], f32)
            ki = bpool.tile([128, CH], mybir.dt.int32)
            kf = bpool.tile([128, CH], f32)
            for nrad in range(8):
                # sin(r * n*pi/R) with range reduction to the LUT's [-pi, pi]
                nc.vector.tensor_scalar(
                    out=sinb[:], in0=r[:],
                    scalar1=float((nrad + 1) / (2.0 * R_MAX)),
                    scalar2=0.5, op0=mult, op1=mybir.AluOpType.add)
                nc.vector.tensor_copy(out=ki[:], in_=sinb[:])
                nc.vector.tensor_copy(out=kf[:], in_=ki[:])
                nc.vector.tensor_sub(out=sinb[:], in0=sinb[:], in1=kf[:])
                nc.vector.tensor_scalar(out=kf[:], in0=sinb[:], scalar1=0.0,
                                        scalar2=None,
                                        op0=mybir.AluOpType.is_lt)
                nc.vector.tensor_add(out=sinb[:], in0=sinb[:], in1=kf[:])
                nc.scalar.activation(out=sinb[:], in_=sinb[:], func=Act.Sin,
                                     scale=2 * math.pi, bias=negpi_ap[:])
                nc.vector.tensor_tensor(out=radial[:, :, nrad], in0=sinb[:],
                                        in1=rse[:], op=mult)

            # per quad: transpose 4 chunks -> [8,512], mm1 (both layers),
            # silu, stash to DRAM.  PE-pipelined: mm1(g-1) after transposes(g).
            pend1 = [None]

            def flush_mm1():
                if pend1[0] is None:
                    return
                radsb, g = pend1[0]
                pend1[0] = None
                w1ps = psW.tile([128, 512], f32, tag="w1")
                nc.tensor.matmul(out=w1ps[:], lhsT=rW1_sb[:],
                                 rhs=radsb[:], start=True, stop=True)
                s1T2 = s1pool.tile([128, 512], bf16, tag="s1T2")
                nc.scalar.activation(out=s1T2[:], in_=w1ps[:], func=Act.Silu)
                nc.sync.dma_start(out=s1T_d[g, :, :], in_=s1T2[:])

            for g in range(CH4):
                radps = psA.tile([8, 512], bf16, tag="mps")
                for q in range(4):
                    cchunk = g * 4 + q
                    nc.tensor.transpose(out=radps[:, q * 128:(q + 1) * 128],
                                        in_=radial[:, cchunk, :],
                                        identity=identb[:])
                radsb = rcpool.tile([8, 512], bf16, tag="radsb")
                if g % 2 == 0:
                    nc.vector.tensor_copy(out=radsb[:], in_=radps[:])
                else:
                    nc.scalar.copy(out=radsb[:], in_=radps[:])
                flush_mm1()
                pend1[0] = (radsb, g)
            flush_mm1()

        # ---------------- layers -------------------------------------------
        lpools = {}
        lpools["s1g"] = ctx.enter_context(tc.tile_pool(name="s1g", bufs=2))
        lpools["hs0"] = ctx.enter_context(tc.tile_pool(name="hs0", bufs=2))
        lpools["hs1"] = ctx.enter_context(
            tc.tile_pool(name="hs1", bufs=8))
        lpools["oh"] = ctx.enter_context(tc.tile_pool(name="oh", bufs=2))
        lpools["B"] = ctx.enter_context(tc.tile_pool(name="B", bufs=3))
        lpools["msg"] = ctx.enter_context(tc.tile_pool(name="msg", bufs=4))
        lpools["post"] = ctx.enter_context(tc.tile_pool(name="post", bufs=2))
        ps_wc = ctx.enter_context(tc.tile_pool(name="pswc", bufs=3, space="PSUM"))
        ps_agg = ctx.enter_context(tc.tile_pool(name="psagg", bufs=2, space="PSUM"))

        tile_of_chunk = []
        for t in range(TILES):
            tile_of_chunk += [t] * (tcs[t + 1] - tcs[t])

        hs_tiles = {}

        def issue_gather(g):
            if g >= NGRP:
                return
            g0 = g * GROUP
            gs = min(GROUP, CHR - g0)
            hst = lpools["hs1"].tile([128, GROUP, 128], bf16,
                                     name="hs1g", tag="hs1")
            nc.gpsimd.dma_gather(
                out_ap=hst[:, :gs, :],
                in_ap=h_full[:, :],
                idxs_ap=idxs_sb[:, g0 * 8:(g0 + gs) * 8],
                num_idxs=gs * 128,
                num_idxs_reg=gs * 128,
                elem_size=128,
                # >1024 idxs overflows the 64-desc/engine packet
                single_packet=False,
            )
            hs_tiles[g] = hst

        def emit_layer(layer):
            if layer == 1:
                for g in range(min(GATHER_AHEAD, NGRP)):
                    issue_gather(g)

            state = dict(agg=None, hs=None, oh=None, s1g=None, pend=None)

            def flush_segsum():
                if state["pend"] is None:
                    return
                c0, kk, msg4, oh_t = state["pend"]
                state["pend"] = None
                for k in range(kk):
                    cc = c0 + k
                    ti = tile_of_chunk[cc]
                    if cc == tcs[ti]:
                        state["agg"] = ps_agg.tile([128, 512], f32,
                                                   name="agg", tag="agg")
                    nc.tensor.matmul(
                        out=state["agg"][:],
                        lhsT=oh_t[:, cc % GROUP, :],
                        rhs=mkap(msg4[:], k * 32, [[128, 16], [1, 32]]),
                        start=(cc == tcs[ti]),
                        stop=(cc == tcs[ti + 1] - 1))
                    if cc == tcs[ti + 1] - 1:
                        emit_tile_post(layer, ti, state["agg"])

            for qd in range(NQ):
                c0 = qd * 4
                kk = min(4, CHR - c0)
                if c0 % GROUP == 0:
                    g = c0 // GROUP
                    g0 = c0
                    gs = min(GROUP, CHR - g0)
                    ng = (gs + 3) // 4
                    state["oh"] = lpools["oh"].tile([128, GROUP, 128], bf16,
                                                    name="ohg", tag="oh")
                    nc.sync.dma_start(
                        out=state["oh"][:, :gs, :],
                        in_=ohT_d[:, g0 * 128:(g0 + gs) * 128])
                    # s1 slice for this layer: [64, ng, 512]
                    state["s1g"] = lpools["s1g"].tile([64, GROUP // 4, 512],
                                                      bf16, name="s1g", tag="s1g")
                    sd = s1T_d[:, :, :]
                    nc.sync.dma_start(
                        out=state["s1g"][:, :ng, :],
                        in_=bass.AP(sd.tensor,
                                    sd.offset + (qd * 128 + layer * 64) * 512,
                                    [[512, 64], [128 * 512, ng], [1, 512]]))
                    # hs source
                    if layer == 0:
                        state["hs"] = lpools["hs0"].tile([128, GROUP, C],
                                                         bf16, name="hs0g", tag="hs0")
                        nc.sync.dma_start(
                            out=state["hs"][:, :gs, :],
                            in_=hs0c_d[:, g0 * C:(g0 + gs) * C])
                    else:
                        state["hs"] = hs_tiles.pop(g)
                        issue_gather(g + GATHER_AHEAD)

                qi = (c0 % GROUP) // 4
                # mm2: w = s1 @ rW2 (l-major, unexpanded) -> [128, kk, 128]
                wcps = ps_wc.tile([128, 4, 128], f32, tag="wc")
                for k in range(kk):
                    nc.tensor.matmul(
                        out=wcps[:, k, :],
                        lhsT=state["s1g"][:, qi, k * 128:(k + 1) * 128],
                        rhs=rW2_sb[:, layer * 128:(layer + 1) * 128],
                        start=True, stop=True)

                # previous quad's seg-sums go behind this quad's mm2s so the
                # PE always has runnable work while the DVE builds msg
                flush_segsum()

                # B = w * h_send   [128, kk, (l,c)]
                B4 = lpools["B"].tile([128, 4, 128], bf16, tag="B")
                wq = wcps[:]
                hq = state["hs"][:]
                W = C if layer == 0 else 128
                hoff = (c0 % GROUP) * W
                nc.vector.tensor_tensor(
                    out=mkap(B4[:], 0, [[128, kk], [1, 128]]),
                    in0=mkap(wq, 0, [[128, kk], [1, 128]]),
                    in1=mkap(hq, hoff, [[W, kk], [0, 4], [1, 32]]),
                    op=mult)

                # msg[e, j, k, c] = B[e, k, (l(j),c)] * sh[e, c0+k, j]
                # j-major across the quad: one TT per l-block covers all 4
                # chunks (amortizes DVE per-op overhead; these run 1x anyway
                # because sh broadcasts over c). Seg-sum reads chunk k via a
                # strided rhs AP.
                msg4 = lpools["msg"].tile([128, 16, 4, 32], bf16, tag="msg4")
                mq = msg4[:]
                sa = sh_all[:]
                # l=0 (sh==1): copy (scalar: DVE is busier)
                nc.scalar.copy(
                    out=mkap(mq, 0, [[32, kk], [1, 32]]),
                    in_=mkap(B4[:], 0, [[128, kk], [1, 32]]))
                for l in (1, 2, 3):
                    j0, jl = L_START[l], L_CNT[l]
                    # l=3 (the widest block) runs on the Pool engine during
                    # layer 0, when the Q7 has no gathers to generate
                    eng = nc.gpsimd if (l == 3 and layer == 0) else nc.vector
                    eng.tensor_tensor(
                        out=mkap(mq, j0 * 128,
                                 [[128, jl], [32, kk], [1, 32]]),
                        in0=mkap(B4[:], l * 32,
                                 [[0, jl], [128, kk], [1, 32]]),
                        in1=mkap(sa, c0 * 16 + j0,
                                 [[1, jl], [16, kk], [0, 32]]),
                        op=mult)
                state["pend"] = (c0, kk, msg4, state["oh"])
            flush_segsum()

        def emit_tile_post(layer, t, agg):
            pp = lpools["post"]
            sq = pp.tile([128, 512], f32, tag="sq")
            nc.scalar.activation(out=sq[:], in_=agg[:], func=Act.Square)
            scal = pp.tile([128, 128], f32, tag="scal")
            sq_cj = sq[:].rearrange("p (j c) -> p c j", j=16)
            for li, (j0, j1) in enumerate(((1, 4), (4, 9), (9, 16))):
                nc.vector.tensor_reduce(
                    out=scal[:, 32 + li * 32:64 + li * 32],
                    in_=sq_cj[:, :, j0:j1],
                    axis=mybir.AxisListType.X, op=mybir.AluOpType.add)
            # sqrt(sumsq + 1e-12) in place for cols 32:128
            nc.scalar.activation(out=scal[:, 32:128], in_=scal[:, 32:128],
                                 func=Act.Sqrt, bias=eps_ap[:])
            nc.vector.tensor_copy(out=scal[:, 0:32], in_=agg[:, 0:32])
            sct = psA.tile([128, 128], f32, tag="mps")
            nc.tensor.transpose(out=sct[:], in_=scal[:], identity=ident[:])
            scT = pp.tile([128, 128], f32, tag="scT")
            nc.scalar.copy(out=scT[:], in_=sct[:])
            hps = psA.tile([128, 32], f32, tag="mps")
            nc.tensor.matmul(out=hps[:], lhsT=scT[:],
                             rhs=Wupd_sb[:, layer * 32:(layer + 1) * 32],
                             start=True, stop=True)
            hsb = pp.tile([128, 32], f32, tag="hsb")
            nc.scalar.activation(out=hsb[:], in_=hps[:], func=Act.Silu)
            hsbb = pp.tile([128, 32], bf16, tag="hsbb")
            nc.vector.tensor_copy(out=hsbb[:], in_=hsb[:])
            nc.sync.dma_start(out=h_own[t * 125:(t + 1) * 125, :],
                              in_=hsbb[:125, :])
            if layer == 1:
                htp = psA.tile([32, 128], f32, tag="mps")
                nc.tensor.transpose(out=htp[:], in_=hsb[:, :], identity=ident[:])
                hT = pp.tile([32, 128], f32, tag="hT")
                nc.vector.tensor_copy(out=hT[:], in_=htp[:])
                r1p = psA.tile([16, 128], f32, tag="mps")
                nc.tensor.matmul(out=r1p[:], lhsT=Wro_sb[:], rhs=hT[:],
                                 start=True, stop=True)
                r1 = pp.tile([16, 128], f32, tag="r1")
                nc.scalar.activation(out=r1[:], in_=r1p[:], func=Act.Silu)
                op_ = psA.tile([1, 128], f32, tag="mps")
                nc.tensor.matmul(out=op_[:], lhsT=Wout_sb[:], rhs=r1[:],
                                 start=True, stop=True)
                osb = pp.tile([1, 128], f32, tag="osb")
                nc.vector.tensor_copy(out=osb[:], in_=op_[:])
                nc.sync.dma_start(out=out_d[t * 125:(t + 1) * 125, :],
                                  in_=osb[:, :125])

        emit_layer(0)
        nc.gpsimd.collective_compute(
            "AllGather", mybir.AluOpType.bypass,
            replica_groups=[list(range(NCORES))],
            ins=[h_own[:, :]], outs=[h_small[:, :]])
        # spread the 32-wide rows into the 256B-aligned gather layout
        hf = h_full[:, :]
        hs_ = h_small[:, :]
        nc.sync.dma_start(
            out=bass.AP(hf.tensor, 0, [[128, N], [1, C]]),
            in_=bass.AP(hs_.tensor, 0, [[C, N], [1, C]]))
        emit_layer(1)

    nc.compile()
    return nc


class TileCtx:
    """thin wrapper so _build doesn't import tile at module scope"""
    def __init__(self, nc, tile_mod):
        self._tc = tile_mod.TileContext(nc)

    def __enter__(self):
        return self._tc.__enter__()

    def __exit__(self, *a):
        return self._tc.__exit__(*a)


# ------------------------------------------------------------------ runner

def kernel(**inputs):
    inputs = {k: np.asarray(v) for k, v in inputs.items()}
    consts, per_core, meta = _prepare(**inputs)
    nc = _build(meta, consts)

    from concourse.bass_utils import run_bass_kernel_spmd
    in_maps = []
    for d in range(NCORES):
        pc = per_core[d]
        in_maps.append(dict(
            xs=pc["xs"], ys=pc["ys"], zs=pc["zs"],
            ohT=pc["ohT"], hs0c=pc["hs0c"], idxs=pc["idxs"],
        ))
    import os
    trace = bool(int(os.environ.get("KBENCH_TRACE", "0")))
    if trace:
        trace = _ensure_ntff_hook()
    res = run_bass_kernel_spmd(nc, in_maps, core_ids=list(range(NCORES)),
                               trace=trace)
    if trace and res.exec_time_ns is not None:
        print(f"HW exec time: {res.exec_time_ns} ns")
        kernel.last_exec_time_ns = res.exec_time_ns
        kernel.last_trace = res.instructions_and_trace
    out = np.concatenate([res.results[d]["out"] for d in range(NCORES)], axis=0)
    full = np.empty_like(out)
    full[meta["perm"]] = out
    return full


kernel.last_exec_time_ns = None
kernel.last_trace = None


def _ensure_ntff_hook():
    """Make trace=True work when the image's antenv lacks axon_hooks."""
    import sys
    import types
    try:
        from antenv.axon_hooks import get_axon_ntff_profile_hook  # noqa: F401
        return True
    except ImportError:
        pass
    try:
        import antenv
        from trn_agent_boot.trn_boot import _ntff_profile_via_ctypes
        hook = _ntff_profile_via_ctypes("/opt/axon/libaxon_pjrt.so")
        m = types.ModuleType("antenv.axon_hooks")
        _state = {"h": hook}
        m.set_axon_ntff_profile_hook = lambda h: _state.__setitem__("h", h)
        m.get_axon_ntff_profile_hook = lambda: _state["h"]
        sys.modules["antenv.axon_hooks"] = m
        antenv.axon_hooks = m
        return hook is not None
    except Exception:
        return False
